# revision 26
# baseline (speedup 1.0000x reference)
"""AffineFlow (FrEIA AllInOneBlock x8) Trainium2 Bass kernel.

Data-parallel over batch: 8192 rows -> 1024 rows per NeuronCore x 8 cores.
On-chip layout is feature-major (features on SBUF partitions, batch on the
free dim), so every layer is out[M=feat,N=batch] = W_T[K,M].T @ x_T[K,N]
with fp32r matmuls (1 cycle/row at N>=256, ~TF32 accuracy) accumulating in
fp32 PSUM.

Host-side parameter folding (tiny, O(F*C^2)):
  scale  = 0.2*softplus(0.5*g)        global affine scale
  WpT_eff = scale[:,None] * Wp.T      folds scale into the soft permutation
  beff    = off @ Wp.T                folds offset into a bias
  woT_eff = 0.1 * wo.T, bo_eff = 0.1*bo   folds the 0.1 subnet output scale
  log|det| = 2*sum_f sum_feat tanh(a1) (device) + sum_f sum_c log(scale) (host)
"""

import os
import sys
import time

sys.path.insert(0, "/opt/trn_rl_repo")

import numpy as np

import concourse.bass as bass  # noqa: F401  (registers engines)
import concourse.mybir as mybir
import concourse.tile as tile
from concourse import bacc
from concourse.bass_utils import run_bass_kernel_spmd

AFT = mybir.ActivationFunctionType
ALU = mybir.AluOpType
f32 = mybir.dt.float32
f32r = mybir.dt.float32r
fp16 = mybir.dt.float16

B, C, F, COUP, NBL = 8192, 512, 8, 1024, 2
D1 = C - C // 2
D2 = C // 2
NCORES = 8
BC = B // NCORES  # 1024 batch rows per core
P = 128
NH = 512  # batch chunk per matmul (one PSUM bank of fp32)
NB_PER_FLOW = 32  # bias columns per flow: 4 beff + 8 b0 + 8 bh0 + 8 bh1 + 4 bo

LAST_EXEC_TIME_NS = None
_NC_CACHE = None


def build_nc():
    global _NC_CACHE
    if _NC_CACHE is not None:
        return _NC_CACHE
    nc = bacc.Bacc("TRN2", target_bir_lowering=False, debug=False)

    zT_d = nc.dram_tensor("zT", [C, BC], f32r, kind="ExternalInput").ap()
    wpT_d = nc.dram_tensor("wpT", [F, C, C], f32r, kind="ExternalInput").ap()
    w0T_d = nc.dram_tensor("w0T", [F, D1, COUP], fp16, kind="ExternalInput").ap()
    whT_d = nc.dram_tensor("whT", [F, NBL, COUP, COUP], fp16, kind="ExternalInput").ap()
    woT_d = nc.dram_tensor("woT", [F, COUP, 2 * D2], fp16, kind="ExternalInput").ap()
    bias_d = nc.dram_tensor("biases", [P, NB_PER_FLOW * F], f32, kind="ExternalInput").ap()
    ones_d = nc.dram_tensor("ones", [P, 1], f32r, kind="ExternalInput").ap()
    zkT_d = nc.dram_tensor("zkT", [C, BC], f32r, kind="ExternalOutput").ap()
    ld_d = nc.dram_tensor("ld", [1, BC], f32, kind="ExternalOutput").ap()

    with tile.TileContext(nc) as tc:
        with (
            tc.tile_pool(name="sb", bufs=1) as sb,
            tc.tile_pool(name="ps", bufs=6, space="PSUM") as ps,
            tc.tile_pool(name="pld", bufs=1, space="PSUM") as pld,
        ):
            # Startup-critical DMAs first: the sync queue issues in order, so
            # the first matmul's inputs (wp0/z0) must head the queue.
            xin = []
            wp0_sb = []
            for k in range(4):
                w = sb.tile([P, C], f32r, tag=f"wp{k}", name=f"wp0_{k}")
                nc.sync.dma_start(out=w, in_=wpT_d[0, k * P : (k + 1) * P, :])
                wp0_sb.append(w)
                xt = sb.tile([P, BC], f32r, tag=f"y{k}", bufs=2, name=f"z{k}")
                nc.sync.dma_start(out=xt, in_=zT_d[k * P : (k + 1) * P, :])
                xin.append(xt)

            bias_sb = sb.tile([P, NB_PER_FLOW * F], f32, tag="bias")
            nc.sync.dma_start(out=bias_sb, in_=bias_d)
            ones_sb = sb.tile([P, 1], f32r, tag="ones")
            nc.sync.dma_start(out=ones_sb, in_=ones_d)
            ld_acc = sb.tile([1, BC], f32, tag="ldacc")
            nc.vector.memset(ld_acc, 0.0)

            # Warm the PE clock (HAM un-throttles after ~3.4us of activity)
            # with throwaway fp32 matmuls while the first weight DMAs land.
            warm_w = sb.tile([P, P], f32, tag="warmw")
            nc.vector.memset(warm_w, 0.0)
            warm_x = sb.tile([P, NH], f32, tag="warmx")
            nc.vector.memset(warm_x, 0.0)
            for i in range(2):
                wpt = ps.tile([P, NH], f32, tag="mm", name=f"warm{i}")
                nc.tensor.matmul(wpt, warm_w, warm_x, start=True, stop=True)

            for f in range(F):
                bb = NB_PER_FLOW * f

                if f == 0:
                    wp_sb = wp0_sb
                else:
                    wp_sb = []
                    for k in range(4):
                        w = sb.tile([P, C], f32r, tag=f"wp{k}", name=f"wp{f}_{k}")
                        nc.sync.dma_start(out=w, in_=wpT_d[f, k * P : (k + 1) * P, :])
                        wp_sb.append(w)
                w0_sb = []
                for k in range(2):
                    w = sb.tile([P, COUP], fp16, tag=f"w0{k}", name=f"w0{f}_{k}")
                    nc.sync.dma_start(out=w, in_=w0T_d[f, k * P : (k + 1) * P, :])
                    w0_sb.append(w)

                # ---- global affine + soft permutation: y = x @ WpT_eff + beff
                y = [
                    sb.tile([P, BC], f32r, tag=f"y{m}", bufs=2, name=f"y{f}_{m}")
                    for m in range(4)
                ]
                y16 = [
                    sb.tile([P, BC], fp16, tag=f"y16_{m}", name=f"y16_{f}_{m}")
                    for m in range(2)
                ]
                for n in range(2):
                    ns = slice(n * NH, (n + 1) * NH)
                    for m in range(4):
                        pt = ps.tile([P, NH], f32, tag="mm", name=f"pwp{f}_{n}_{m}")
                        for k in range(4):
                            nc.tensor.matmul(
                                pt,
                                wp_sb[k][:, m * P : (m + 1) * P],
                                xin[k][:, ns],
                                start=(k == 0),
                                stop=(k == 3),
                            )
                        if m < 2:
                            # fp16 twin of x1 for the fp16 w0 matmul, on DVE
                            # so it runs parallel to the ACT f32r-carry copy
                            nc.vector.tensor_scalar(
                                out=y16[m][:, ns],
                                in0=pt,
                                scalar1=bias_sb[:, bb + m : bb + m + 1],
                                scalar2=None,
                                op0=ALU.add,
                            )
                        nc.scalar.activation(
                            y[m][:, ns], pt, AFT.Identity,
                            bias=bias_sb[:, bb + m : bb + m + 1],
                        )
                        if f == F - 1 and m < 2:
                            # final x1 halves: store as soon as produced, on
                            # the otherwise-idle GpSimd queue
                            nc.gpsimd.dma_start(
                                out=zkT_d[m * P : (m + 1) * P, ns],
                                in_=y[m][:, ns],
                            )

                # ---- subnet layer 0: h0 = relu(y1 @ w0T + b0)
                # h0/h1 are fp16: the wh matmuls then take 2-byte LDWEIGHTS
                # (97ns, fully hidden) instead of f32r's 4-byte (187ns, which
                # costs +11ns/MM); h2 stays f32r to feed the f32r wo matmul.
                h = [
                    sb.tile([P, BC], fp16, tag=f"hA{m}", name=f"h0_{f}_{m}")
                    for m in range(8)
                ]
                for n in range(2):
                    ns = slice(n * NH, (n + 1) * NH)
                    for m in range(8):
                        pt = ps.tile([P, NH], f32, tag="mm", name=f"pw0{f}_{n}_{m}")
                        for k in range(2):
                            nc.tensor.matmul(
                                pt,
                                w0_sb[k][:, m * P : (m + 1) * P],
                                y16[k][:, ns],
                                start=(k == 0),
                                stop=(k == 1),
                            )
                        nc.scalar.activation(
                            h[m][:, ns], pt, AFT.Relu,
                            bias=bias_sb[:, bb + 4 + m : bb + 5 + m],
                        )

                # ---- subnet hidden layers: h = relu(h @ whT + bh), x2
                for l in range(NBL):
                    hout = [
                        sb.tile(
                            [P, BC], fp16,
                            tag=(f"hB{m}" if l == 0 else f"hA{m}"),
                            name=f"h{l + 1}_{f}_{m}",
                        )
                        for m in range(8)
                    ]
                    for half in range(2):
                        whw = []
                        for k in range(8):
                            w = sb.tile(
                                [P, NH], fp16,
                                tag=(f"whL{k}" if half == 0 else f"whH{k}"),
                                name=f"wh{f}_{l}_{half}_{k}",
                            )
                            nc.sync.dma_start(
                                out=w,
                                in_=whT_d[
                                    f, l, k * P : (k + 1) * P,
                                    half * NH : (half + 1) * NH,
                                ],
                            )
                            whw.append(w)
                        for n in range(2):
                            ns = slice(n * NH, (n + 1) * NH)
                            for m in range(4):
                                gm = half * 4 + m
                                pt = ps.tile(
                                    [P, NH], f32, tag="mm",
                                    name=f"pwh{f}_{l}_{n}_{gm}",
                                )
                                for k in range(8):
                                    nc.tensor.matmul(
                                        pt,
                                        whw[k][:, m * P : (m + 1) * P],
                                        h[k][:, ns],
                                        start=(k == 0),
                                        stop=(k == 7),
                                    )
                                bcol = bb + 12 + l * 8 + gm
                                nc.vector.tensor_scalar(
                                    out=hout[gm][:, ns],
                                    in0=pt,
                                    scalar1=bias_sb[:, bcol : bcol + 1],
                                    scalar2=0.0,
                                    op0=ALU.add,
                                    op1=ALU.max,
                                )
                    h = hout

                # ---- output layer + coupling
                wo_sb = []
                for k in range(8):
                    w = sb.tile([P, 2 * D2], fp16, tag=f"wo{k}", name=f"wo{f}_{k}")
                    nc.sync.dma_start(out=w, in_=woT_d[f, k * P : (k + 1) * P, :])
                    wo_sb.append(w)

                t1 = [
                    sb.tile([P, BC], f32r, tag=f"t1_{m}", name=f"t1_{f}_{m}")
                    for m in range(2)
                ]
                y2n = [
                    sb.tile([P, BC], f32r, tag=f"y2n{m}", name=f"y2n{f}_{m}")
                    for m in range(2)
                ]
                Et = [
                    sb.tile([P, BC], f32r, tag=f"E{m}", name=f"E{f}_{m}")
                    for m in range(2)
                ]
                for n in range(2):
                    ns = slice(n * NH, (n + 1) * NH)
                    # s-half: t1 = tanh(a1 + bo1)
                    for m in range(2):
                        pt = ps.tile([P, NH], f32, tag="mm", name=f"pa1{f}_{n}_{m}")
                        for k in range(8):
                            nc.tensor.matmul(
                                pt,
                                wo_sb[k][:, m * P : (m + 1) * P],
                                h[k][:, ns],
                                start=(k == 0),
                                stop=(k == 7),
                            )
                        nc.scalar.activation(
                            t1[m][:, ns], pt, AFT.Tanh,
                            bias=bias_sb[:, bb + 28 + m : bb + 29 + m],
                        )
                    # t-half kept in PSUM, consumed by the fused coupling add
                    pa2 = []
                    for m in range(2):
                        pt = ps.tile([P, NH], f32, tag="mm", name=f"pa2{f}_{n}_{m}")
                        for k in range(8):
                            nc.tensor.matmul(
                                pt,
                                wo_sb[k][:, (2 + m) * P : (3 + m) * P],
                                h[k][:, ns],
                                start=(k == 0),
                                stop=(k == 7),
                            )
                        pa2.append(pt)
                    # log-det partial: sum over features of tanh (x2 host-side)
                    lps = pld.tile([1, NH], f32, tag=f"ldps{n}", name=f"lps{f}_{n}")
                    nc.tensor.matmul(lps, ones_sb, t1[0][:, ns], start=True, stop=False)
                    nc.tensor.matmul(lps, ones_sb, t1[1][:, ns], start=False, stop=True)
                    nc.vector.tensor_tensor(
                        out=ld_acc[:, ns], in0=ld_acc[:, ns], in1=lps, op=ALU.add
                    )
                    # E = exp(2*tanh) into its own tile so ACT needn't wait
                    # for the PE ld-matmul's read of t1 (write-after-read);
                    # then y2n = (a2 + bo2) + y2*E
                    for m in range(2):
                        nc.scalar.activation(
                            Et[m][:, ns], t1[m][:, ns], AFT.Exp, scale=2.0
                        )
                        nc.vector.tensor_tensor(
                            out=y2n[m][:, ns],
                            in0=y[2 + m][:, ns],
                            in1=Et[m][:, ns],
                            op=ALU.mult,
                        )
                        nc.vector.scalar_tensor_tensor(
                            out=y2n[m][:, ns],
                            in0=pa2[m],
                            scalar=bias_sb[:, bb + 30 + m : bb + 31 + m],
                            in1=y2n[m][:, ns],
                            op0=ALU.add,
                            op1=ALU.add,
                        )
                        if f == F - 1:
                            # stream the final x2 halves out as they finish
                            nc.gpsimd.dma_start(
                                out=zkT_d[(2 + m) * P : (3 + m) * P, ns],
                                in_=y2n[m][:, ns],
                            )

                xin = [y[0], y[1], y2n[0], y2n[1]]

            nc.sync.dma_start(out=ld_d, in_=ld_acc)

    nc.compile()
    _NC_CACHE = nc
    return nc


def preprocess(z0, Wp, g, off, w0, b0, wh, bh, wo, bo):
    """Host-side parameter folding; returns (in_maps, ld_const)."""
    z0, Wp, g, off = (np.asarray(a, np.float32) for a in (z0, Wp, g, off))
    w0, b0, wh, bh, wo, bo = (
        np.asarray(a, np.float32) for a in (w0, b0, wh, bh, wo, bo)
    )
    # scale = 0.2 * softplus(0.5*g), stable softplus
    hg = 0.5 * g.astype(np.float64)
    scale = 0.2 * (np.logaddexp(0.0, hg))
    ld_const = float(np.log(scale).sum())
    scale = scale.astype(np.float32)

    wpT = np.ascontiguousarray(scale[:, :, None] * Wp.transpose(0, 2, 1))
    beff = np.einsum("fj,fij->fi", off, Wp).astype(np.float32)
    w0T = np.ascontiguousarray(w0.transpose(0, 2, 1)).astype(np.float16)
    whT = np.ascontiguousarray(wh.transpose(0, 1, 3, 2)).astype(np.float16)
    woT = np.ascontiguousarray(0.1 * wo.transpose(0, 2, 1)).astype(np.float16)
    bo_eff = 0.1 * bo

    bias_pack = np.zeros((P, NB_PER_FLOW * F), np.float32)
    for f in range(F):
        base = NB_PER_FLOW * f
        bias_pack[:, base : base + 4] = beff[f].reshape(4, P).T
        bias_pack[:, base + 4 : base + 12] = b0[f].reshape(8, P).T
        bias_pack[:, base + 12 : base + 20] = bh[f, 0].reshape(8, P).T
        bias_pack[:, base + 20 : base + 28] = bh[f, 1].reshape(8, P).T
        bias_pack[:, base + 28 : base + 32] = bo_eff[f].reshape(4, P).T

    in_maps = []
    for c in range(NCORES):
        zT = np.ascontiguousarray(z0[c * BC : (c + 1) * BC, :].T)
        in_maps.append(
            dict(zT=zT, wpT=wpT, w0T=w0T, whT=whT, woT=woT, biases=bias_pack,
                 ones=np.ones((P, 1), np.float32))
        )
    return in_maps, ld_const


def postprocess(results, ld_const):
    zk = np.empty((B, C), np.float32)
    ld = np.empty((B,), np.float32)
    for c in range(NCORES):
        zk[c * BC : (c + 1) * BC] = results[c]["zkT"].T
        ld[c * BC : (c + 1) * BC] = 2.0 * results[c]["ld"].reshape(-1) + ld_const
    return zk, ld


def _run_in_maps(in_maps):
    # The axon trace path needs antenv.axon_hooks, which this container's
    # antenv stub lacks - force the plain execute path.
    os.environ.pop("BASS_TRACE", None)
    nc = build_nc()
    res = run_bass_kernel_spmd(nc, in_maps, list(range(NCORES)))
    return res.results, res.exec_time_ns


def _subproc_entry(in_path, out_path):
    data = np.load(in_path)
    shared = {k: data[k] for k in ("wpT", "w0T", "whT", "woT", "biases", "ones")}
    in_maps = [dict(shared, zT=data[f"zT{c}"]) for c in range(NCORES)]
    results, _ = _run_in_maps(in_maps)
    np.savez(
        out_path,
        **{f"{k}{c}": v for c, r in enumerate(results) for k, v in r.items()},
    )


def _run_in_subprocess(in_maps):
    """Fresh process = fresh axon/NRT client; clears wedged-device state that
    in-process retries cannot."""
    import subprocess
    import tempfile

    tmpdir = tempfile.mkdtemp()
    in_path = os.path.join(tmpdir, "in.npz")
    out_path = os.path.join(tmpdir, "out.npz")
    shared = {k: in_maps[0][k] for k in ("wpT", "w0T", "whT", "woT", "biases", "ones")}
    np.savez(in_path, **shared, **{f"zT{c}": in_maps[c]["zT"] for c in range(NCORES)})
    subprocess.run(
        [sys.executable, os.path.abspath(__file__), "--subproc", in_path, out_path],
        check=True,
        timeout=3600,
    )
    data = np.load(out_path)
    return [
        {"zkT": data[f"zkT{c}"], "ld": data[f"ld{c}"]} for c in range(NCORES)
    ]


def _results_equal(r1, r2):
    return all(
        np.array_equal(r1[c][k], r2[c][k]) for c in range(NCORES) for k in ("zkT", "ld")
    )


def kernel(z0, Wp, g, off, w0, b0, wh, bh, wo, bo):
    global LAST_EXEC_TIME_NS
    in_maps, ld_const = preprocess(z0, Wp, g, off, w0, b0, wh, bh, wo, bo)
    # Two failure modes seen on this setup, both from flaky device state left
    # by earlier crashed processes:
    #  - hard: NRT_EXEC_UNIT_UNRECOVERABLE / mesh desync (raises; does NOT
    #    clear within a process)
    #  - silent: a run "succeeds" but one shard comes back slightly corrupted
    # The kernel is bit-deterministic when healthy, so run twice and compare;
    # any exception or mismatch falls through to fresh subprocesses.
    last_exc = None
    for attempt in range(2):
        try:
            results, LAST_EXEC_TIME_NS = _run_in_maps(in_maps)
            results2, _ = _run_in_maps(in_maps)
            if _results_equal(results, results2):
                return postprocess(results, ld_const)
            last_exc = RuntimeError("nondeterministic device results")
        except Exception as e:  # noqa: BLE001
            last_exc = e
        time.sleep(10)
    for attempt in range(2):
        try:
            results = _run_in_subprocess(in_maps)
            return postprocess(results, ld_const)
        except Exception as e:  # noqa: BLE001
            last_exc = e
            time.sleep(30)
    raise last_exc


if __name__ == "__main__" and len(sys.argv) == 4 and sys.argv[1] == "--subproc":
    _subproc_entry(sys.argv[2], sys.argv[3])



# revision 27
# speedup vs baseline: 1.0252x; 1.0252x over previous
"""AffineFlow (FrEIA AllInOneBlock x8) Trainium2 Bass kernel.

Data-parallel over batch: 8192 rows -> 1024 rows per NeuronCore x 8 cores.
On-chip layout is feature-major (features on SBUF partitions, batch on the
free dim), so every layer is out[M=feat,N=batch] = W_T[K,M].T @ x_T[K,N]
with fp32r matmuls (1 cycle/row at N>=256, ~TF32 accuracy) accumulating in
fp32 PSUM.

Host-side parameter folding (tiny, O(F*C^2)):
  scale  = 0.2*softplus(0.5*g)        global affine scale
  WpT_eff = scale[:,None] * Wp.T      folds scale into the soft permutation
  beff    = off @ Wp.T                folds offset into a bias
  woT_eff = 0.1 * wo.T, bo_eff = 0.1*bo   folds the 0.1 subnet output scale
  log|det| = 2*sum_f sum_feat tanh(a1) (device) + sum_f sum_c log(scale) (host)
"""

import os
import sys
import time

sys.path.insert(0, "/opt/trn_rl_repo")

import numpy as np

import concourse.bass as bass  # noqa: F401  (registers engines)
import concourse.mybir as mybir
import concourse.tile as tile
from concourse import bacc
from concourse.bass_utils import run_bass_kernel_spmd

AFT = mybir.ActivationFunctionType
ALU = mybir.AluOpType
f32 = mybir.dt.float32
f32r = mybir.dt.float32r
fp16 = mybir.dt.float16

B, C, F, COUP, NBL = 8192, 512, 8, 1024, 2
D1 = C - C // 2
D2 = C // 2
NCORES = 8
BC = B // NCORES  # 1024 batch rows per core
P = 128
NH = 512  # batch chunk per matmul (one PSUM bank of fp32)
NB_PER_FLOW = 32  # bias columns per flow: 4 beff + 8 b0 + 8 bh0 + 8 bh1 + 4 bo

LAST_EXEC_TIME_NS = None
_NC_CACHE = None


def build_nc():
    global _NC_CACHE
    if _NC_CACHE is not None:
        return _NC_CACHE
    nc = bacc.Bacc("TRN2", target_bir_lowering=False, debug=False)

    zT_d = nc.dram_tensor("zT", [C, BC], f32r, kind="ExternalInput").ap()
    wpT_d = nc.dram_tensor("wpT", [F, C, C], f32r, kind="ExternalInput").ap()
    w0T_d = nc.dram_tensor("w0T", [F, D1, COUP], fp16, kind="ExternalInput").ap()
    whT_d = nc.dram_tensor("whT", [F, NBL, COUP, COUP], fp16, kind="ExternalInput").ap()
    woT_d = nc.dram_tensor("woT", [F, COUP, 2 * D2], fp16, kind="ExternalInput").ap()
    bias_d = nc.dram_tensor("biases", [P, NB_PER_FLOW * F], f32, kind="ExternalInput").ap()
    ones_d = nc.dram_tensor("ones", [P, 1], f32r, kind="ExternalInput").ap()
    zkT_d = nc.dram_tensor("zkT", [C, BC], f32r, kind="ExternalOutput").ap()
    ld_d = nc.dram_tensor("ld", [1, BC], f32, kind="ExternalOutput").ap()

    with tile.TileContext(nc) as tc:
        with (
            tc.tile_pool(name="sb", bufs=1) as sb,
            tc.tile_pool(name="ps", bufs=6, space="PSUM") as ps,
            tc.tile_pool(name="pld", bufs=1, space="PSUM") as pld,
        ):
            # Startup-critical DMAs first: the sync queue issues in order, so
            # the first matmul's inputs (wp0/z0) must head the queue.
            xin = []
            wp0_sb = []
            for k in range(4):
                w = sb.tile([P, C], f32r, tag=f"wp{k}", name=f"wp0_{k}")
                nc.sync.dma_start(out=w, in_=wpT_d[0, k * P : (k + 1) * P, :])
                wp0_sb.append(w)
                xt = sb.tile([P, BC], f32r, tag=f"y{k}", bufs=2, name=f"z{k}")
                nc.sync.dma_start(out=xt, in_=zT_d[k * P : (k + 1) * P, :])
                xin.append(xt)

            bias_sb = sb.tile([P, NB_PER_FLOW * F], f32, tag="bias")
            nc.sync.dma_start(out=bias_sb, in_=bias_d)
            ones_sb = sb.tile([P, 1], f32r, tag="ones")
            nc.sync.dma_start(out=ones_sb, in_=ones_d)
            ld_acc = sb.tile([1, BC], f32, tag="ldacc")
            nc.vector.memset(ld_acc, 0.0)

            # Warm the PE clock (HAM un-throttles after ~3.4us of activity)
            # with throwaway fp32 matmuls while the first weight DMAs land.
            warm_w = sb.tile([P, P], f32, tag="warmw")
            nc.vector.memset(warm_w, 0.0)
            warm_x = sb.tile([P, NH], f32, tag="warmx")
            nc.vector.memset(warm_x, 0.0)
            for i in range(2):
                wpt = ps.tile([P, NH], f32, tag="mm", name=f"warm{i}")
                nc.tensor.matmul(wpt, warm_w, warm_x, start=True, stop=True)

            for f in range(F):
                bb = NB_PER_FLOW * f

                if f == 0:
                    wp_sb = wp0_sb
                else:
                    wp_sb = []
                    for k in range(4):
                        w = sb.tile([P, C], f32r, tag=f"wp{k}", name=f"wp{f}_{k}")
                        nc.sync.dma_start(out=w, in_=wpT_d[f, k * P : (k + 1) * P, :])
                        wp_sb.append(w)
                w0_sb = []
                for k in range(2):
                    w = sb.tile([P, COUP], fp16, tag=f"w0{k}", name=f"w0{f}_{k}")
                    nc.sync.dma_start(out=w, in_=w0T_d[f, k * P : (k + 1) * P, :])
                    w0_sb.append(w)

                # ---- global affine + soft permutation: y = x @ WpT_eff + beff
                y = [
                    sb.tile([P, BC], f32r, tag=f"y{m}", bufs=2, name=f"y{f}_{m}")
                    for m in range(4)
                ]
                y16 = [
                    sb.tile([P, BC], fp16, tag=f"y16_{m}", name=f"y16_{f}_{m}")
                    for m in range(2)
                ]
                for n in range(2):
                    ns = slice(n * NH, (n + 1) * NH)
                    for m in range(4):
                        pt = ps.tile([P, NH], f32, tag="mm", name=f"pwp{f}_{n}_{m}")
                        for k in range(4):
                            nc.tensor.matmul(
                                pt,
                                wp_sb[k][:, m * P : (m + 1) * P],
                                xin[k][:, ns],
                                start=(k == 0),
                                stop=(k == 3),
                            )
                        if m < 2:
                            # fp16 twin of x1 for the fp16 w0 matmul, on DVE
                            # so it runs parallel to the ACT f32r-carry copy
                            nc.vector.tensor_scalar(
                                out=y16[m][:, ns],
                                in0=pt,
                                scalar1=bias_sb[:, bb + m : bb + m + 1],
                                scalar2=None,
                                op0=ALU.add,
                            )
                        nc.scalar.activation(
                            y[m][:, ns], pt, AFT.Identity,
                            bias=bias_sb[:, bb + m : bb + m + 1],
                        )
                        if f == F - 1 and m < 2:
                            # final x1 halves: store as soon as produced, on
                            # the otherwise-idle GpSimd queue
                            nc.gpsimd.dma_start(
                                out=zkT_d[m * P : (m + 1) * P, ns],
                                in_=y[m][:, ns],
                            )

                # ---- subnet layer 0: h0 = relu(y1 @ w0T + b0)
                # The whole subnet (w0/wh/wo weights and h activations) runs
                # fp16: 2-byte LDWEIGHTS (97ns, hidden) vs f32r's 4-byte
                # (187ns, +11ns/MM), and the 0.1*tanh squash keeps the
                # rounding contribution at ~2e-4. Only the x-carry path
                # (Wp, y tiles, t1/E, coupling) stays f32r.
                h = [
                    sb.tile([P, BC], fp16, tag=f"hA{m}", name=f"h0_{f}_{m}")
                    for m in range(8)
                ]
                for n in range(2):
                    ns = slice(n * NH, (n + 1) * NH)
                    for m in range(8):
                        pt = ps.tile([P, NH], f32, tag="mm", name=f"pw0{f}_{n}_{m}")
                        for k in range(2):
                            nc.tensor.matmul(
                                pt,
                                w0_sb[k][:, m * P : (m + 1) * P],
                                y16[k][:, ns],
                                start=(k == 0),
                                stop=(k == 1),
                            )
                        nc.scalar.activation(
                            h[m][:, ns], pt, AFT.Relu,
                            bias=bias_sb[:, bb + 4 + m : bb + 5 + m],
                        )

                # ---- subnet hidden layers: h = relu(h @ whT + bh), x2
                for l in range(NBL):
                    hout = [
                        sb.tile(
                            [P, BC], fp16,
                            tag=(f"hB{m}" if l == 0 else f"hA{m}"),
                            name=f"h{l + 1}_{f}_{m}",
                        )
                        for m in range(8)
                    ]
                    for half in range(2):
                        whw = []
                        for k in range(8):
                            w = sb.tile(
                                [P, NH], fp16,
                                tag=(f"whL{k}" if half == 0 else f"whH{k}"),
                                name=f"wh{f}_{l}_{half}_{k}",
                            )
                            nc.sync.dma_start(
                                out=w,
                                in_=whT_d[
                                    f, l, k * P : (k + 1) * P,
                                    half * NH : (half + 1) * NH,
                                ],
                            )
                            whw.append(w)
                        for n in range(2):
                            ns = slice(n * NH, (n + 1) * NH)
                            for m in range(4):
                                gm = half * 4 + m
                                pt = ps.tile(
                                    [P, NH], f32, tag="mm",
                                    name=f"pwh{f}_{l}_{n}_{gm}",
                                )
                                for k in range(8):
                                    nc.tensor.matmul(
                                        pt,
                                        whw[k][:, m * P : (m + 1) * P],
                                        h[k][:, ns],
                                        start=(k == 0),
                                        stop=(k == 7),
                                    )
                                bcol = bb + 12 + l * 8 + gm
                                nc.vector.tensor_scalar(
                                    out=hout[gm][:, ns],
                                    in0=pt,
                                    scalar1=bias_sb[:, bcol : bcol + 1],
                                    scalar2=0.0,
                                    op0=ALU.add,
                                    op1=ALU.max,
                                )
                    h = hout

                # ---- output layer + coupling
                wo_sb = []
                for k in range(8):
                    w = sb.tile([P, 2 * D2], fp16, tag=f"wo{k}", name=f"wo{f}_{k}")
                    nc.sync.dma_start(out=w, in_=woT_d[f, k * P : (k + 1) * P, :])
                    wo_sb.append(w)

                t1 = [
                    sb.tile([P, BC], f32r, tag=f"t1_{m}", name=f"t1_{f}_{m}")
                    for m in range(2)
                ]
                y2n = [
                    sb.tile([P, BC], f32r, tag=f"y2n{m}", name=f"y2n{f}_{m}")
                    for m in range(2)
                ]
                Et = [
                    sb.tile([P, BC], f32r, tag=f"E{m}", name=f"E{f}_{m}")
                    for m in range(2)
                ]
                for n in range(2):
                    ns = slice(n * NH, (n + 1) * NH)
                    # s-half: t1 = tanh(a1 + bo1)
                    for m in range(2):
                        pt = ps.tile([P, NH], f32, tag="mm", name=f"pa1{f}_{n}_{m}")
                        for k in range(8):
                            nc.tensor.matmul(
                                pt,
                                wo_sb[k][:, m * P : (m + 1) * P],
                                h[k][:, ns],
                                start=(k == 0),
                                stop=(k == 7),
                            )
                        nc.scalar.activation(
                            t1[m][:, ns], pt, AFT.Tanh,
                            bias=bias_sb[:, bb + 28 + m : bb + 29 + m],
                        )
                    # t-half kept in PSUM, consumed by the fused coupling add
                    pa2 = []
                    for m in range(2):
                        pt = ps.tile([P, NH], f32, tag="mm", name=f"pa2{f}_{n}_{m}")
                        for k in range(8):
                            nc.tensor.matmul(
                                pt,
                                wo_sb[k][:, (2 + m) * P : (3 + m) * P],
                                h[k][:, ns],
                                start=(k == 0),
                                stop=(k == 7),
                            )
                        pa2.append(pt)
                    # log-det partial: sum over features of tanh (x2 host-side)
                    lps = pld.tile([1, NH], f32, tag=f"ldps{n}", name=f"lps{f}_{n}")
                    nc.tensor.matmul(lps, ones_sb, t1[0][:, ns], start=True, stop=False)
                    nc.tensor.matmul(lps, ones_sb, t1[1][:, ns], start=False, stop=True)
                    nc.vector.tensor_tensor(
                        out=ld_acc[:, ns], in0=ld_acc[:, ns], in1=lps, op=ALU.add
                    )
                    # E = exp(2*tanh) into its own tile so ACT needn't wait
                    # for the PE ld-matmul's read of t1 (write-after-read);
                    # then y2n = (a2 + bo2) + y2*E
                    for m in range(2):
                        nc.scalar.activation(
                            Et[m][:, ns], t1[m][:, ns], AFT.Exp, scale=2.0
                        )
                        nc.vector.tensor_tensor(
                            out=y2n[m][:, ns],
                            in0=y[2 + m][:, ns],
                            in1=Et[m][:, ns],
                            op=ALU.mult,
                        )
                        nc.vector.scalar_tensor_tensor(
                            out=y2n[m][:, ns],
                            in0=pa2[m],
                            scalar=bias_sb[:, bb + 30 + m : bb + 31 + m],
                            in1=y2n[m][:, ns],
                            op0=ALU.add,
                            op1=ALU.add,
                        )
                        if f == F - 1:
                            # stream the final x2 halves out as they finish
                            nc.gpsimd.dma_start(
                                out=zkT_d[(2 + m) * P : (3 + m) * P, ns],
                                in_=y2n[m][:, ns],
                            )

                xin = [y[0], y[1], y2n[0], y2n[1]]

            nc.sync.dma_start(out=ld_d, in_=ld_acc)

    nc.compile()
    _NC_CACHE = nc
    return nc


def preprocess(z0, Wp, g, off, w0, b0, wh, bh, wo, bo):
    """Host-side parameter folding; returns (in_maps, ld_const)."""
    z0, Wp, g, off = (np.asarray(a, np.float32) for a in (z0, Wp, g, off))
    w0, b0, wh, bh, wo, bo = (
        np.asarray(a, np.float32) for a in (w0, b0, wh, bh, wo, bo)
    )
    # scale = 0.2 * softplus(0.5*g), stable softplus
    hg = 0.5 * g.astype(np.float64)
    scale = 0.2 * (np.logaddexp(0.0, hg))
    ld_const = float(np.log(scale).sum())
    scale = scale.astype(np.float32)

    wpT = np.ascontiguousarray(scale[:, :, None] * Wp.transpose(0, 2, 1))
    beff = np.einsum("fj,fij->fi", off, Wp).astype(np.float32)
    w0T = np.ascontiguousarray(w0.transpose(0, 2, 1)).astype(np.float16)
    whT = np.ascontiguousarray(wh.transpose(0, 1, 3, 2)).astype(np.float16)
    woT = np.ascontiguousarray(0.1 * wo.transpose(0, 2, 1)).astype(np.float16)
    bo_eff = 0.1 * bo

    bias_pack = np.zeros((P, NB_PER_FLOW * F), np.float32)
    for f in range(F):
        base = NB_PER_FLOW * f
        bias_pack[:, base : base + 4] = beff[f].reshape(4, P).T
        bias_pack[:, base + 4 : base + 12] = b0[f].reshape(8, P).T
        bias_pack[:, base + 12 : base + 20] = bh[f, 0].reshape(8, P).T
        bias_pack[:, base + 20 : base + 28] = bh[f, 1].reshape(8, P).T
        bias_pack[:, base + 28 : base + 32] = bo_eff[f].reshape(4, P).T

    in_maps = []
    for c in range(NCORES):
        zT = np.ascontiguousarray(z0[c * BC : (c + 1) * BC, :].T)
        in_maps.append(
            dict(zT=zT, wpT=wpT, w0T=w0T, whT=whT, woT=woT, biases=bias_pack,
                 ones=np.ones((P, 1), np.float32))
        )
    return in_maps, ld_const


def postprocess(results, ld_const):
    zk = np.empty((B, C), np.float32)
    ld = np.empty((B,), np.float32)
    for c in range(NCORES):
        zk[c * BC : (c + 1) * BC] = results[c]["zkT"].T
        ld[c * BC : (c + 1) * BC] = 2.0 * results[c]["ld"].reshape(-1) + ld_const
    return zk, ld


def _run_in_maps(in_maps):
    # The axon trace path needs antenv.axon_hooks, which this container's
    # antenv stub lacks - force the plain execute path.
    os.environ.pop("BASS_TRACE", None)
    nc = build_nc()
    res = run_bass_kernel_spmd(nc, in_maps, list(range(NCORES)))
    return res.results, res.exec_time_ns


def _subproc_entry(in_path, out_path):
    data = np.load(in_path)
    shared = {k: data[k] for k in ("wpT", "w0T", "whT", "woT", "biases", "ones")}
    in_maps = [dict(shared, zT=data[f"zT{c}"]) for c in range(NCORES)]
    results, _ = _run_in_maps(in_maps)
    np.savez(
        out_path,
        **{f"{k}{c}": v for c, r in enumerate(results) for k, v in r.items()},
    )


def _run_in_subprocess(in_maps):
    """Fresh process = fresh axon/NRT client; clears wedged-device state that
    in-process retries cannot."""
    import subprocess
    import tempfile

    tmpdir = tempfile.mkdtemp()
    in_path = os.path.join(tmpdir, "in.npz")
    out_path = os.path.join(tmpdir, "out.npz")
    shared = {k: in_maps[0][k] for k in ("wpT", "w0T", "whT", "woT", "biases", "ones")}
    np.savez(in_path, **shared, **{f"zT{c}": in_maps[c]["zT"] for c in range(NCORES)})
    subprocess.run(
        [sys.executable, os.path.abspath(__file__), "--subproc", in_path, out_path],
        check=True,
        timeout=3600,
    )
    data = np.load(out_path)
    return [
        {"zkT": data[f"zkT{c}"], "ld": data[f"ld{c}"]} for c in range(NCORES)
    ]


def _results_equal(r1, r2):
    return all(
        np.array_equal(r1[c][k], r2[c][k]) for c in range(NCORES) for k in ("zkT", "ld")
    )


def kernel(z0, Wp, g, off, w0, b0, wh, bh, wo, bo):
    global LAST_EXEC_TIME_NS
    in_maps, ld_const = preprocess(z0, Wp, g, off, w0, b0, wh, bh, wo, bo)
    # Two failure modes seen on this setup, both from flaky device state left
    # by earlier crashed processes:
    #  - hard: NRT_EXEC_UNIT_UNRECOVERABLE / mesh desync (raises; does NOT
    #    clear within a process)
    #  - silent: a run "succeeds" but one shard comes back slightly corrupted
    # The kernel is bit-deterministic when healthy, so run twice and compare;
    # any exception or mismatch falls through to fresh subprocesses.
    last_exc = None
    for attempt in range(2):
        try:
            results, LAST_EXEC_TIME_NS = _run_in_maps(in_maps)
            results2, _ = _run_in_maps(in_maps)
            if _results_equal(results, results2):
                return postprocess(results, ld_const)
            last_exc = RuntimeError("nondeterministic device results")
        except Exception as e:  # noqa: BLE001
            last_exc = e
        time.sleep(10)
    for attempt in range(2):
        try:
            results = _run_in_subprocess(in_maps)
            return postprocess(results, ld_const)
        except Exception as e:  # noqa: BLE001
            last_exc = e
            time.sleep(30)
    raise last_exc


if __name__ == "__main__" and len(sys.argv) == 4 and sys.argv[1] == "--subproc":
    _subproc_entry(sys.argv[2], sys.argv[3])

